# revision 1
# baseline (speedup 1.0000x reference)
"""Trainium2 Bass kernel for nn_AttentiveGatingv2 (moe_routing).

Reference computation (shapes hardcoded):
  x: [64, 96, 207, 64] -> take last 6 timesteps -> per-(b,n) token:
  z = proj(x_k); qkv = in_proj(z); 4-head attention over the 6 steps;
  out-proj; mean over steps; fc to 8 experts; softmax -> [64, 207, 8].

Host-side algebraic fusion (verified vs reference):
  W_eff = in_proj_w @ proj_w  (96x64), b_eff = in_proj_w@proj_b + in_proj_b
  (q-rows pre-scaled by 1/sqrt(8)); since mean-over-steps commutes with the
  linear out-proj/fc, post-attention collapses to
  logits = G @ (sum_j wbar_j * v_j) + g_b  with  G = fc_w@out_w/6,
  g_b = fc_w@out_b + fc_b,  wbar_j = sum_i softmax_j(scores)_ij.

Layout strategy: host pre-slices the 6 needed timesteps (1/16 of x), casts
to bf16, packs them feature-major with an appended ones-row so the single
PE matmul  qkv[tokens,96] = x_aug[65,tokens].T @ W_aug[65,96]  lands
token-major in PSUM (fp32) with bias included.  Attention math runs on
VectorE in bf16 (validated ~5e-4 rel-to-max on the final softmax output),
with 128-token tiles processed in groups of 4 so the small softmax/context
ops amortize instruction overhead; expert-logit matmuls run per pair of
tiles through one PE transpose + a block-diagonal G.  8 NeuronCores
data-parallel over batch; no cross-device communication.

Measured on trn2 (8 cores, via axon): HW exec ~58.5us/core, rel err 5.4e-4.
Progression: 160us (first correct fp32) -> 90 (drop serializing hacks) ->
73 (bf16) -> 62 (pair batching) -> 58.5 (quad batching, load stagger,
scalar-queue const loads, deeper work pool). GQ=6 measured
worse (60.4us: batched ops stall behind 6 evacuations), so GQ=4 stands.
"""

import numpy as np
import ml_dtypes

import concourse.bass as bass
import concourse.mybir as mybir
import concourse.tile as tile
from concourse.bacc import Bacc
from concourse.bass_utils import run_bass_kernel_spmd

F32 = mybir.dt.float32
BF16 = mybir.dt.bfloat16
NP_BF16 = ml_dtypes.bfloat16

# problem dims
B, T, NTOK, C = 64, 96, 207, 64
D, H, HD, K = 32, 4, 8, 6
E = 8
NCORES = 8

# per-core dims
B_SH = B // NCORES            # 8
S = B_SH * NTOK               # 1656 tokens per core
P = 128
NT = (S + P - 1) // P         # 13 tiles
S_PAD = NT * P                # 1664
CA = C + 1                    # 65: channels + ones row
E3 = 3 * D                    # 96
KK = K * K                    # 36
DA = D + 1                    # 33


def _build_module():
    nc = Bacc()

    xt = nc.dram_tensor("xt", [CA, K, S_PAD], BF16, kind="ExternalInput")
    wa = nc.dram_tensor("wa", [CA, E3], BF16, kind="ExternalInput")
    # cf packs block-diag G2_aug [66,16] | identity [128,128] (fp32)
    cf = nc.dram_tensor("cf", [P, 2 * E + P], F32, kind="ExternalInput")
    # out[p, t, e]: token (t*128+p); host reassembles. This layout keeps the
    # single final store one-descriptor-per-partition contiguous.
    out = nc.dram_tensor("out", [P, NT, E], F32, kind="ExternalOutput")

    AF = mybir.ActivationFunctionType
    AX = mybir.AxisListType

    def apv(t, dims, extra_offset=0):
        # custom AP over tile t: keep t's partition dim, replace free dims
        return bass.AP(
            tensor=t.tensor,
            offset=t.offset + extra_offset,
            ap=[list(t.ap[0])] + [list(d) for d in dims],
        )

    GQ = 4                                                   # tiles per group
    groups = [(t, min(GQ, NT - t)) for t in range(0, NT, GQ)]  # (base, size)

    with tile.TileContext(nc) as tc:
        with (
            tc.tile_pool(name="singles", bufs=1) as singles,
            tc.tile_pool(name="xload", bufs=4) as xload,
            tc.tile_pool(name="work", bufs=3) as work,
            tc.tile_pool(name="psum", bufs=2, space="PSUM") as psum,
        ):
            # DMA issue costs ~0.8us each on a sequencer: put the two
            # constant loads on the Scalar HWDGE queue (issues in parallel
            # with the Sync queue issuing x loads), and load x per tile-PAIR
            # to halve the issue count.
            wa_sb = singles.tile([CA, E3], BF16)
            nc.scalar.dma_start(out=wa_sb, in_=wa[:, :])
            cf_sb = singles.tile([P, 2 * E + P], F32)
            nc.scalar.dma_start(out=cf_sb, in_=cf[:, :])
            ga2_sb = cf_sb[0:2 * DA, 0:2 * E]
            id_sb = cf_sb[:, 2 * E:]

            xg_tiles = []
            xg_dmas = []
            for gi, (tg, g) in enumerate(groups):
                xg_sb = xload.tile([CA, K, GQ * P], BF16, name=f"xg_sb{gi}",
                                   tag="xg")
                xd = nc.sync.dma_start(
                    out=xg_sb[:, :, 0:g * P],
                    in_=xt[:, :, tg * P:(tg + g) * P])
                xg_tiles.append(xg_sb)
                xg_dmas.append(xd)

            out_sb = singles.tile([P, NT, E], F32)

            qkv_first = []
            for gi, (tg, g) in enumerate(groups):
                if gi >= 2:
                    # stagger loads ~2 groups ahead of compute so early tiles
                    # don't round-robin behind all the loads
                    tile.add_dep_helper(xg_dmas[gi].ins,
                                        qkv_first[gi - 2].ins,
                                        sync=True, reason="load stagger")
                qk_sb = work.tile([P, GQ, K, E3], BF16)
                tmp = work.tile([P, GQ, KK, D], BF16)
                for u in range(g):
                    t = tg + u
                    xt_sb = xg_tiles[gi][:, :, u * P:(u + 1) * P]

                    # ---- qkv: 6 matmuls (bf16 in, fp32 psum) ----
                    # [P, 8, 128] = exactly 2 PSUM banks so slots are
                    # bank-aligned (1.5-bank slots would share a bank)
                    qkv_ps = psum.tile([P, 8, 128], F32, tag="qkv_ps", bufs=2,
                                       name="qkv_ps")
                    for i in range(K):
                        mm = nc.tensor.matmul(
                            out=qkv_ps[:, i, 0:E3],
                            lhsT=xt_sb[:, i, :],
                            rhs=wa_sb[:, :],
                            start=True,
                            stop=True,
                        )
                        if i == 0 and u == 0:
                            qkv_first.append(mm)

                    # ---- evacuate q,k,v to SBUF as bf16 on ScalarE ----
                    nc.scalar.copy(out=qk_sb[:, u], in_=qkv_ps[:, 0:K, 0:E3])

                    # ---- scores tmp[i,j,(h,c)] = q[i,(hc)] * k[j,(hc)] ----
                    # (TensorTensor allows at most 3 free AP dims: (h,c) is
                    # kept merged, and the i/j broadcasts force per-tile muls)
                    off = u * K * E3
                    q_ap = apv(qk_sb, [[E3, K], [0, K], [1, D]], off)
                    k_ap = apv(qk_sb, [[0, K], [E3, K], [1, D]], off + D)
                    tm_out = apv(tmp, [[D, KK], [1, D]], u * KK * D)
                    nc.vector.tensor_mul(tm_out, q_ap, k_ap)

                # ---- batched over the pair from here on ----
                gKK = g * KK
                # scores[(t,i,j), h] = sum_c tmp via add tree: tensor_reduce
                # costs input-elems (2304/pair @1x) vs 1152+576+576 for the
                # tree (TT cost follows output elems)
                s1 = work.tile([P, GQ, KK, H, 4], BF16)
                a_ap = apv(tmp, [[D, gKK], [HD, H], [1, 4]])
                b_ap = apv(tmp, [[D, gKK], [HD, H], [1, 4]], 4)
                o_ap = apv(s1, [[16, gKK], [4, H], [1, 4]])
                nc.vector.tensor_add(o_ap, a_ap, b_ap)
                s2 = work.tile([P, GQ, KK, H, 2], BF16)
                a_ap = apv(s1, [[16, gKK], [4, H], [1, 2]])
                b_ap = apv(s1, [[16, gKK], [4, H], [1, 2]], 2)
                o_ap = apv(s2, [[8, gKK], [2, H], [1, 2]])
                nc.vector.tensor_add(o_ap, a_ap, b_ap)
                scores = work.tile([P, GQ, KK, H], F32)
                a_ap = apv(s2, [[8, gKK], [2, H]])
                b_ap = apv(s2, [[8, gKK], [2, H]], 1)
                o_ap = apv(scores, [[H, gKK], [1, H]])
                nc.vector.tensor_add(o_ap, a_ap, b_ap)

                # ---- softmax over j (scores pre-scaled, |s|<1.5) ----
                es = work.tile([P, GQ, K, K, H], BF16)
                nc.scalar.activation(out=es[:, 0:g], in_=scores[:, 0:g],
                                     func=AF.Exp)
                zs = work.tile([P, GQ, K, H], F32)
                es_jred = apv(es, [[K * H, g * K], [1, H], [H, K]])  # [(t,i),h,j]
                zs_o = apv(zs, [[1, g * K * H]])
                nc.vector.reduce_sum(out=zs_o, in_=es_jred, axis=AX.X)
                rs = work.tile([P, GQ, K, H], F32)
                nc.vector.reciprocal(rs[:, 0:g], zs[:, 0:g])
                # attn[(t,i),j,h] = es * rs
                attn = work.tile([P, GQ, K, K, H], BF16)
                es_ap = apv(es, [[K * H, g * K], [H, K], [1, H]])
                rs_ap = apv(rs, [[H, g * K], [0, K], [1, H]])
                at_o = apv(attn, [[K * H, g * K], [H, K], [1, H]])
                nc.vector.tensor_mul(at_o, es_ap, rs_ap)

                # ---- wbar[t,(j,h)] = sum_i attn[t,i,(jh)] ----
                wbar = work.tile([P, GQ, K, H], F32)
                at_ap = apv(attn, [[K * K * H, g], [1, K * H], [K * H, K]])
                wb_o = apv(wbar, [[K * H, g], [1, K * H]])
                nc.vector.reduce_sum(out=wb_o, in_=at_ap, axis=AX.X)

                # ---- ct[(t,j),h,c] = wbar[(t,jh)] * v[t,j,(hc)] ----
                ct = work.tile([P, GQ, K, H, HD], BF16)
                wb_ap = apv(wbar, [[1, g * K * H], [0, HD]])
                v_ap = apv(qk_sb, [[K * E3, g], [E3, K], [1, D]], 2 * D)
                ct_o = apv(ct, [[1, g * K * D]])
                nc.vector.tensor_mul(ct_o, wb_ap, v_ap)

                # ---- ctxbar[t,(h,c)] = sum_j ct; col D set to 1 ----
                cb = work.tile([P, GQ, DA], F32)
                ct_ap = apv(ct, [[K * D, g], [1, D], [D, K]])  # [t,(hc),j]
                cb_o = apv(cb, [[DA, g], [1, D]])
                nc.vector.reduce_sum(out=cb_o, in_=ct_ap, axis=AX.X)
                nc.vector.memset(cb[:, 0:g, D:DA], 1.0)

                # ---- logits: per-PAIR transpose (<=128 out partitions) +
                # block-diag matmul ----
                el = work.tile([P, GQ, E], F32)
                for p0 in range(0, g, 2):
                    gp = min(2, g - p0)
                    ctT_ps = psum.tile([2 * DA, P], F32, bufs=2,
                                       tag="ctT_ps", name="ctT_ps")
                    nc.tensor.transpose(ctT_ps[0:gp * DA, :],
                                        cb[:, p0:p0 + gp, :], id_sb)
                    ctT_sb = work.tile([2 * DA, P], F32, tag="ctT_sb",
                                       name="ctT_sb")
                    nc.scalar.copy(out=ctT_sb[0:gp * DA],
                                   in_=ctT_ps[0:gp * DA])

                    log_ps = psum.tile([P, 2 * E], F32, bufs=2,
                                       tag="log_ps", name="log_ps")
                    nc.tensor.matmul(
                        out=log_ps[:, 0:gp * E],
                        lhsT=ctT_sb[0:gp * DA, :],
                        rhs=ga2_sb[0:gp * DA, 0:gp * E],
                        start=True, stop=True,
                    )
                    nc.scalar.activation(out=el[:, p0:p0 + gp],
                                         in_=log_ps[:, 0:gp * E],
                                         func=AF.Exp)

                # ---- final softmax over 8 experts (batched) ----
                zf = work.tile([P, GQ], F32)
                nc.vector.reduce_sum(out=zf[:, 0:g], in_=el[:, 0:g],
                                     axis=AX.X)
                rf = work.tile([P, GQ], F32)
                nc.vector.reciprocal(rf[:, 0:g], zf[:, 0:g])
                rf_ap = apv(rf, [[1, g], [0, E]])
                nc.vector.tensor_mul(out_sb[:, tg:tg + g, :],
                                     el[:, 0:g], rf_ap)
                # per-group store on the software-DGE path (idle sequencer,
                # overlaps compute; final store is then tiny)
                nc.gpsimd.dma_start(out=out[:, tg:tg + g, :],
                                    in_=out_sb[:, tg:tg + g, :])

    nc.finalize()
    return nc


_NC = None


def _get_module():
    global _NC
    if _NC is None:
        _NC = _build_module()
    return _NC


def _host_prep(x, proj_w, proj_b, in_proj_w, in_proj_b, out_w, out_b, fc_w, fc_b):
    scale = np.float32(1.0 / np.sqrt(HD))
    w_eff = (in_proj_w @ proj_w).astype(np.float32)          # [96, 64]
    b_eff = (in_proj_w @ proj_b + in_proj_b).astype(np.float32)
    w_eff[0:D] *= scale
    b_eff[0:D] *= scale
    wa = np.concatenate([w_eff.T, b_eff[None, :]], axis=0)   # [65, 96]
    wa = np.ascontiguousarray(wa).astype(NP_BF16)

    g = (fc_w @ out_w / np.float32(K)).astype(np.float32)    # [8, 32]
    g_b = (fc_w @ out_b + fc_b).astype(np.float32)
    ga = np.concatenate([g.T, g_b[None, :]], axis=0)         # [33, 8]

    cf = np.zeros((P, 2 * E + P), dtype=np.float32)
    cf[0:DA, 0:E] = ga
    cf[DA:2 * DA, E:2 * E] = ga                               # block-diag
    cf[:, 2 * E:] = np.eye(P, dtype=np.float32)

    # x: [B, T, N, C] -> last K steps -> per-core [65, K, S_PAD] feature-major
    xk = x[:, T - K:, :, :]                                  # [B, K, N, C]
    in_maps = []
    for core in range(NCORES):
        xc = xk[core * B_SH:(core + 1) * B_SH]               # [8, K, N, C]
        # -> [C, K, b, N] -> [C, K, S]
        xc = np.transpose(xc, (3, 1, 0, 2)).reshape(C, K, S)
        xtc = np.ones((CA, K, S_PAD), dtype=NP_BF16)
        xtc[0:C, :, 0:S] = xc.astype(NP_BF16)
        xtc[0:C, :, S:] = 0
        in_maps.append({"xt": xtc, "wa": wa, "cf": cf})
    return in_maps


def kernel(x, proj_w, proj_b, in_proj_w, in_proj_b, out_w, out_b, fc_w, fc_b,
           _trace=False):
    in_maps = _host_prep(np.asarray(x, dtype=np.float32),
                         np.asarray(proj_w, dtype=np.float32),
                         np.asarray(proj_b, dtype=np.float32),
                         np.asarray(in_proj_w, dtype=np.float32),
                         np.asarray(in_proj_b, dtype=np.float32),
                         np.asarray(out_w, dtype=np.float32),
                         np.asarray(out_b, dtype=np.float32),
                         np.asarray(fc_w, dtype=np.float32),
                         np.asarray(fc_b, dtype=np.float32))
    nc = _get_module()
    res = run_bass_kernel_spmd(nc, in_maps, core_ids=list(range(NCORES)),
                               trace=_trace)
    outs = []
    for core in range(NCORES):
        oc = res.results[core]["out"]                        # [P, NT, E]
        oc = oc.transpose(1, 0, 2).reshape(S_PAD, E)[:S]
        oc = oc.reshape(B_SH, NTOK, E)
        outs.append(oc)
    full = np.concatenate(outs, axis=0)                      # [64, 207, 8]
    if _trace:
        kernel._last_exec_time_ns = res.exec_time_ns
        kernel._last_profile = res.profile_json
    return full.astype(np.float32)



# revision 4
# speedup vs baseline: 1.2473x; 1.2473x over previous
"""Trainium2 Bass kernel for nn_AttentiveGatingv2 (moe_routing).

Reference computation (shapes hardcoded):
  x: [64, 96, 207, 64] -> take last 6 timesteps -> per-(b,n) token:
  z = proj(x_k); qkv = in_proj(z); 4-head attention over the 6 steps;
  out-proj; mean over steps; fc to 8 experts; softmax -> [64, 207, 8].

Host-side algebraic fusion (validated vs reference in fp-faithful sim):
  W_eff = in_proj_w @ proj_w  (96x64), b_eff folded via an appended ones
  row (q-rows pre-scaled by 1/sqrt(8)).  Since mean-over-steps commutes
  with the linear out-proj/fc, and  ctxbar^{hc} = sum_j wbar_j^h v_j^{hc},
  the whole post-attention stack collapses INTO the qkv matmul weights:
    logits_e = sum_{j,h} wbar_j^h * ghv_j^{h,e} ,
    ghv columns = (G_h W_v_h) x_aug  with  G = fc_w@out_w/6.
  So the single PE matmul per (tile, step) produces q(32) | k(32) | ghv(32)
  and NO post-attention matmul/transpose is needed; the expert logits are
  one bf16 2x-mode DVE mul + an add-tree/reduce.  exp(g_b) is folded in as
  a broadcast multiply before the final softmax.

v2 layout/engine strategy (from perfetto analysis of v1 @ 58.1us: VectorE
38us busy of 58, strided reduces 1.4-1.7x model, reciprocal 730ns, 14 PE
round-trips mid-chain):
  - scores/softmax math on VectorE in bf16 with ALL hot ops in 2x mode
    (step-1 innermost APs; broadcasts only on middle dims).
  - reduce_sum replaced by bf16 add-trees where the X-dim was strided.
  - reciprocal -> reciprocal_approx_fast (~5x).
  - ghv evacuated PSUM->SBUF transposed to [e, j, h] so the logits mul
    runs 2x with contiguous reduce.
  - software pipelining ACROSS tile groups {4,4,5}: DVE emission order
    A0 A1 C0 A2 C1 E0 C2 E1 E2 (A=scores, C=softmax+logits, E=final
    softmax) so the engine-FIFO never stalls on ScalarE exp handoffs.
  - x loaded in 6 chunks (2,2,2,2,2,3 tiles) from a group-blocked dram
    layout [65, 13, 6, 128]; later loads gated on compute progress.
8 NeuronCores data-parallel over batch; no cross-device communication.

Measured on trn2 (8 cores, via axon): v1 58.1us -> see test log for v2.
"""

import numpy as np
import ml_dtypes

import concourse.bass as bass
import concourse.mybir as mybir
import concourse.tile as tile
from concourse.bacc import Bacc
from concourse.bass_utils import run_bass_kernel_spmd

F32 = mybir.dt.float32
BF16 = mybir.dt.bfloat16
NP_BF16 = ml_dtypes.bfloat16

# problem dims
B, T, NTOK, C = 64, 96, 207, 64
D, H, HD, K = 32, 4, 8, 6
E = 8
NCORES = 8

# per-core dims
B_SH = B // NCORES            # 8
S = B_SH * NTOK               # 1656 tokens per core
P = 128
NT = (S + P - 1) // P         # 13 tiles
S_PAD = NT * P                # 1664
CA = C + 1                    # 65: channels + ones row
E3 = 3 * D                    # 96 matmul output cols: q(32)|k(32)|ghv(32)
KK = K * K                    # 36
GQ = 5                        # max tiles per group (groups are 4,4,5)

GROUPS = [(0, 4), (4, 4), (8, 5)]
LOADS = [(0, 2), (2, 2), (4, 2), (6, 2), (8, 2), (10, 3)]


def _build_module():
    nc = Bacc()

    xt = nc.dram_tensor("xt", [CA, NT, K, P], BF16, kind="ExternalInput")
    wa = nc.dram_tensor("wa", [CA, E3], BF16, kind="ExternalInput")
    cf = nc.dram_tensor("cf", [P, E], F32, kind="ExternalInput")  # exp(g_b)
    # out[p, t, e]: token (t*128+p); host reassembles.
    out = nc.dram_tensor("out", [P, NT, E], F32, kind="ExternalOutput")

    AF = mybir.ActivationFunctionType
    AX = mybir.AxisListType

    def apv(t, dims, extra_offset=0):
        # custom AP over tile t: keep t's partition dim, replace free dims
        return bass.AP(
            tensor=t.tensor,
            offset=t.offset + extra_offset,
            ap=[list(t.ap[0])] + [list(d) for d in dims],
        )

    with tile.TileContext(nc) as tc:
        with (
            tc.tile_pool(name="singles", bufs=1) as singles,
            tc.tile_pool(name="xload", bufs=1) as xload,
            tc.tile_pool(name="work", bufs=3) as work,
            tc.tile_pool(name="psum", bufs=3, space="PSUM") as psum,
        ):
            # constants on the Scalar HWDGE queue (parallel with Sync's x
            # loads); dummy exp right after to pull the ACT table load off
            # the critical path.
            wa_sb = singles.tile([CA, E3], BF16)
            nc.scalar.dma_start(out=wa_sb, in_=wa[:, :])
            cf_sb = singles.tile([P, E], F32)
            nc.scalar.dma_start(out=cf_sb, in_=cf[:, :])
            dum = singles.tile([P, 1], F32)
            nc.scalar.activation(out=dum, in_=cf_sb[:, 0:1], func=AF.Exp)

            xg_tiles = []
            xg_dmas = []
            for li, (ts, nl) in enumerate(LOADS):
                xg_sb = xload.tile([CA, nl, K, P], BF16, name=f"xg{li}")
                xd = nc.sync.dma_start(out=xg_sb, in_=xt[:, ts:ts + nl])
                xg_tiles.append(xg_sb)
                xg_dmas.append(xd)

            out_sb = singles.tile([P, NT, E], F32)

            # ---- per-group tiles (allocated once per group via names) ----
            def group_tiles(gi):
                t = {}
                t["qk"] = work.tile([P, GQ, K, 2 * D], BF16, name="qk")
                t["gb"] = work.tile([P, GQ, E, K, H], BF16, name="gb")
                t["tmp"] = work.tile([P, GQ, KK, D], BF16, name="tmp")
                t["s1"] = work.tile([P, GQ, KK, H, 4], BF16, name="s1")
                t["s2"] = work.tile([P, GQ, KK, H, 2], BF16, name="s2")
                t["sc"] = work.tile([P, GQ, KK, H], F32, name="sc")
                t["es"] = work.tile([P, GQ, K, K, H], BF16, name="es")
                t["zt1"] = work.tile([P, GQ, K, 3, H], BF16, name="zt1")
                t["zs2"] = work.tile([P, GQ, K, H], BF16, name="zs2")
                t["zs"] = work.tile([P, GQ, K, H], F32, name="zs")
                t["rs32"] = work.tile([P, GQ, K, H], F32, name="rs32")
                t["rs16"] = work.tile([P, GQ, K, H], BF16, name="rs16")
                t["at"] = work.tile([P, GQ, K, K, H], BF16, name="at")
                t["wb1"] = work.tile([P, GQ, 3, K, H], BF16, name="wb1")
                t["wb2"] = work.tile([P, GQ, K, H], BF16, name="wb2")
                t["wbar"] = work.tile([P, GQ, K, H], BF16, name="wbar")
                t["lg1"] = work.tile([P, GQ, E, K, H], BF16, name="lg1")
                t["lgt"] = work.tile([P, GQ, E, 12], BF16, name="lgt")
                t["lg"] = work.tile([P, GQ, E], F32, name="lg")
                t["el"] = work.tile([P, GQ, E], F32, name="el")
                t["el2"] = work.tile([P, GQ, E], F32, name="el2")
                t["zf"] = work.tile([P, GQ], F32, name="zf")
                t["rf"] = work.tile([P, GQ], F32, name="rf")
                return t

            gts = [group_tiles(gi) for gi in range(len(GROUPS))]
            first_mm = {}           # global tile idx -> first matmul inst

            def tile_load(tix):
                for li, (ts, nl) in enumerate(LOADS):
                    if ts <= tix < ts + nl:
                        return li, tix - ts
                raise AssertionError

            def emit_mm_evac(gi):
                tg, g = GROUPS[gi]
                t = gts[gi]
                for u in range(g):
                    tix = tg + u
                    li, lidx = tile_load(tix)
                    qkv_ps = psum.tile([P, 8, P], F32, tag="qkv_ps", bufs=3,
                                       name="qkv_ps")
                    for i in range(K):
                        mm = nc.tensor.matmul(
                            out=qkv_ps[:, i, 0:E3],
                            lhsT=xg_tiles[li][:, lidx, i, :],
                            rhs=wa_sb[:, :],
                            start=True,
                            stop=True,
                        )
                        if i == 0 and tix not in first_mm:
                            first_mm[tix] = mm
                    # evac q,k as-is; ghv transposed to [e, j, h] so the
                    # logits mul runs in 2x mode with a contiguous reduce
                    nc.scalar.copy(out=t["qk"][:, u],
                                   in_=qkv_ps[:, 0:K, 0:2 * D])
                    nc.scalar.copy(
                        out=apv(t["gb"], [[K * H, E], [H, K], [1, H]],
                                u * E * K * H),
                        in_=apv(qkv_ps, [[1, E], [P, K], [E, H]], 2 * D))

            def emit_A(gi):
                # scores: tmp mul per tile + bf16 add tree (s1,s2 2x; s3 1x)
                tg, g = GROUPS[gi]
                t = gts[gi]
                gKK = g * KK
                for u in range(g):
                    off = u * K * 2 * D
                    q_ap = apv(t["qk"], [[2 * D, K], [0, K], [1, D]], off)
                    k_ap = apv(t["qk"], [[0, K], [2 * D, K], [1, D]], off + D)
                    tm_o = apv(t["tmp"], [[D, KK], [1, D]], u * KK * D)
                    nc.vector.tensor_mul(tm_o, q_ap, k_ap)
                a = apv(t["tmp"], [[D, gKK], [HD, H], [1, 4]])
                b = apv(t["tmp"], [[D, gKK], [HD, H], [1, 4]], 4)
                o = apv(t["s1"], [[16, gKK], [4, H], [1, 4]])
                nc.vector.tensor_add(o, a, b)
                a = apv(t["s1"], [[16, gKK], [4, H], [1, 2]])
                b = apv(t["s1"], [[16, gKK], [4, H], [1, 2]], 2)
                o = apv(t["s2"], [[8, gKK], [2, H], [1, 2]])
                nc.vector.tensor_add(o, a, b)
                a = apv(t["s2"], [[8, gKK], [2, H]])
                b = apv(t["s2"], [[8, gKK], [2, H]], 1)
                o = apv(t["sc"], [[H, gKK], [1, H]])
                nc.vector.tensor_add(o, a, b)

            def emit_B(gi):
                tg, g = GROUPS[gi]
                t = gts[gi]
                nc.scalar.activation(out=t["es"][:, 0:g], in_=t["sc"][:, 0:g],
                                     func=AF.Exp)

            def emit_C(gi):
                tg, g = GROUPS[gi]
                t = gts[gi]
                gK = g * K
                # zs = sum_j es via bf16 tree (reduce would read stride-4)
                a = apv(t["es"], [[24, gK], [4, 3], [1, H]])
                b = apv(t["es"], [[24, gK], [4, 3], [1, H]], 12)
                o = apv(t["zt1"], [[12, gK], [4, 3], [1, H]])
                nc.vector.tensor_add(o, a, b)
                a = apv(t["zt1"], [[12, gK], [1, H]])
                b = apv(t["zt1"], [[12, gK], [1, H]], 4)
                o = apv(t["zs2"], [[4, gK], [1, H]])
                nc.vector.tensor_add(o, a, b)
                a = apv(t["zs2"], [[4, gK], [1, H]])
                b = apv(t["zt1"], [[12, gK], [1, H]], 8)
                o = apv(t["zs"], [[4, gK], [1, H]])
                nc.vector.tensor_add(o, a, b)
                n = gK * H
                nc.vector.reciprocal_approx_fast(
                    out=apv(t["rs32"], [[1, n]]), in_=apv(t["zs"], [[1, n]]))
                nc.vector.tensor_copy(out=apv(t["rs16"], [[1, n]]),
                                      in_=apv(t["rs32"], [[1, n]]))
                # attn = es * rs (2x: broadcast only on middle dim)
                a = apv(t["es"], [[24, gK], [4, K], [1, H]])
                b = apv(t["rs16"], [[4, gK], [0, K], [1, H]])
                o = apv(t["at"], [[24, gK], [4, K], [1, H]])
                nc.vector.tensor_mul(o, a, b)
                # wbar = sum_i attn via bf16 tree
                a = apv(t["at"], [[144, g], [24, 3], [1, K * H]])
                b = apv(t["at"], [[144, g], [24, 3], [1, K * H]], 72)
                o = apv(t["wb1"], [[72, g], [24, 3], [1, K * H]])
                nc.vector.tensor_add(o, a, b)
                a = apv(t["wb1"], [[72, g], [1, K * H]])
                b = apv(t["wb1"], [[72, g], [1, K * H]], 24)
                o = apv(t["wb2"], [[24, g], [1, K * H]])
                nc.vector.tensor_add(o, a, b)
                a = apv(t["wb2"], [[24, g], [1, K * H]])
                b = apv(t["wb1"], [[72, g], [1, K * H]], 48)
                o = apv(t["wbar"], [[24, g], [1, K * H]])
                nc.vector.tensor_add(o, a, b)
                # logits partial: lg1[g, e, (j,h)] = wbar[g, (j,h)] * ghv
                a = apv(t["wbar"], [[24, g], [0, E], [1, K * H]])
                b = apv(t["gb"], [[K * H * E, g], [K * H, E], [1, K * H]])
                o = apv(t["lg1"], [[K * H * E, g], [K * H, E], [1, K * H]])
                nc.vector.tensor_mul(o, a, b)
                # half-tree then contiguous reduce over 12
                a = apv(t["lg1"], [[K * H * E, g], [K * H, E], [1, 12]])
                b = apv(t["lg1"], [[K * H * E, g], [K * H, E], [1, 12]], 12)
                o = apv(t["lgt"], [[12 * E, g], [12, E], [1, 12]])
                nc.vector.tensor_add(o, a, b)
                nc.vector.reduce_sum(
                    out=apv(t["lg"], [[E, g], [1, E]]),
                    in_=apv(t["lgt"], [[12 * E, g], [12, E], [1, 12]]),
                    axis=AX.X)

            def emit_D(gi):
                tg, g = GROUPS[gi]
                t = gts[gi]
                nc.scalar.activation(out=t["el"][:, 0:g], in_=t["lg"][:, 0:g],
                                     func=AF.Exp)

            def emit_E(gi):
                tg, g = GROUPS[gi]
                t = gts[gi]
                ebg = apv(cf_sb, [[0, g], [1, E]])
                nc.vector.tensor_mul(t["el2"][:, 0:g], t["el"][:, 0:g], ebg)
                nc.vector.reduce_sum(out=t["zf"][:, 0:g],
                                     in_=t["el2"][:, 0:g], axis=AX.X)
                nc.vector.reciprocal_approx_fast(out=t["rf"][:, 0:g],
                                                 in_=t["zf"][:, 0:g])
                rf_ap = apv(t["rf"], [[1, g], [0, E]])
                nc.vector.tensor_mul(out_sb[:, tg:tg + g, :],
                                     t["el2"][:, 0:g], rf_ap)
                nc.gpsimd.dma_start(out=out[:, tg:tg + g, :],
                                    in_=out_sb[:, tg:tg + g, :])

            # ---- software-pipelined emission ----
            emit_mm_evac(0)
            emit_mm_evac(1)
            emit_A(0)
            emit_A(1)
            emit_B(0)
            emit_mm_evac(2)
            emit_C(0)
            emit_A(2)
            emit_B(1)
            emit_C(1)
            emit_D(0)
            emit_E(0)
            emit_B(2)
            emit_C(2)
            emit_D(1)
            emit_E(1)
            emit_D(2)
            emit_E(2)

            # later loads wait on compute progress so early tiles don't
            # round-robin behind all the loads
            for li in range(2, len(LOADS)):
                gate_tile = LOADS[li - 2][0]
                tile.add_dep_helper(xg_dmas[li].ins, first_mm[gate_tile].ins,
                                    sync=True, reason="load stagger")

    nc.finalize()
    return nc


_NC = None


def _get_module():
    global _NC
    if _NC is None:
        _NC = _build_module()
    return _NC


def _host_prep(x, proj_w, proj_b, in_proj_w, in_proj_b, out_w, out_b, fc_w, fc_b):
    scale = np.float32(1.0 / np.sqrt(HD))
    w_eff = (in_proj_w @ proj_w).astype(np.float32)          # [96, 64]
    b_eff = (in_proj_w @ proj_b + in_proj_b).astype(np.float32)
    w_eff[0:D] *= scale
    b_eff[0:D] *= scale
    w_aug = np.concatenate([w_eff, b_eff[:, None]], axis=1)  # [96, 65]

    G = (fc_w @ out_w / np.float32(K)).astype(np.float32)    # [8, 32]
    g_b = (fc_w @ out_b + fc_b).astype(np.float32)

    wa = np.zeros((CA, E3), dtype=np.float32)
    wa[:, 0:2 * D] = w_aug[0:2 * D].T                        # q | k
    for h in range(H):
        wv_h = w_aug[2 * D + HD * h:2 * D + HD * (h + 1)]    # [8, 65]
        G_h = G[:, HD * h:HD * (h + 1)]                      # [8(e), 8(c)]
        wa[:, 2 * D + E * h:2 * D + E * (h + 1)] = wv_h.T @ G_h.T
    wa = np.ascontiguousarray(wa).astype(NP_BF16)

    cf = np.broadcast_to(np.exp(g_b).astype(np.float32)[None, :],
                         (P, E)).copy()

    # x: [B, T, N, C] -> last K steps -> per-core [CA, NT, K, P]
    xk = x[:, T - K:, :, :]                                  # [B, K, N, C]
    in_maps = []
    for core in range(NCORES):
        xc = xk[core * B_SH:(core + 1) * B_SH]               # [8, K, N, C]
        xc = np.transpose(xc, (3, 1, 0, 2)).reshape(C, K, S)
        xp = np.zeros((C, K, S_PAD), dtype=np.float32)
        xp[:, :, 0:S] = xc
        xp = xp.reshape(C, K, NT, P).transpose(0, 2, 1, 3)   # [C, NT, K, P]
        xtc = np.empty((CA, NT, K, P), dtype=NP_BF16)
        xtc[0:C] = xp.astype(NP_BF16)
        xtc[C] = 1
        in_maps.append({"xt": xtc, "wa": wa, "cf": cf})
    return in_maps


def kernel(x, proj_w, proj_b, in_proj_w, in_proj_b, out_w, out_b, fc_w, fc_b,
           _trace=False):
    in_maps = _host_prep(np.asarray(x, dtype=np.float32),
                         np.asarray(proj_w, dtype=np.float32),
                         np.asarray(proj_b, dtype=np.float32),
                         np.asarray(in_proj_w, dtype=np.float32),
                         np.asarray(in_proj_b, dtype=np.float32),
                         np.asarray(out_w, dtype=np.float32),
                         np.asarray(out_b, dtype=np.float32),
                         np.asarray(fc_w, dtype=np.float32),
                         np.asarray(fc_b, dtype=np.float32))
    nc = _get_module()
    res = run_bass_kernel_spmd(nc, in_maps, core_ids=list(range(NCORES)),
                               trace=_trace)
    outs = []
    for core in range(NCORES):
        oc = res.results[core]["out"]                        # [P, NT, E]
        oc = oc.transpose(1, 0, 2).reshape(S_PAD, E)[:S]
        oc = oc.reshape(B_SH, NTOK, E)
        outs.append(oc)
    full = np.concatenate(outs, axis=0)                      # [64, 207, 8]
    if _trace:
        kernel._last_exec_time_ns = res.exec_time_ns
        kernel._last_profile = res.profile_json
    return full.astype(np.float32)


# revision 6
# speedup vs baseline: 1.2580x; 1.0086x over previous
"""Trainium2 Bass kernel for nn_AttentiveGatingv2 (moe_routing).

Reference computation (shapes hardcoded):
  x: [64, 96, 207, 64] -> take last 6 timesteps -> per-(b,n) token:
  z = proj(x_k); qkv = in_proj(z); 4-head attention over the 6 steps;
  out-proj; mean over steps; fc to 8 experts; softmax -> [64, 207, 8].

Host-side algebraic fusion (validated vs reference in fp-faithful sim):
  W_eff = in_proj_w @ proj_w  (96x64), b_eff folded via an appended ones
  row (q-rows pre-scaled by 1/sqrt(8)).  Since mean-over-steps commutes
  with the linear out-proj/fc, and  ctxbar^{hc} = sum_j wbar_j^h v_j^{hc},
  the whole post-attention stack collapses INTO the qkv matmul weights:
    logits_e = sum_{j,h} wbar_j^h * ghv_j^{h,e} ,
    ghv columns = (G_h W_v_h) x_aug  with  G = fc_w@out_w/6.
  So the single PE matmul per (tile, step) produces q(32) | k(32) | ghv(32)
  and NO post-attention matmul/transpose is needed; the expert logits are
  one bf16 2x-mode DVE mul + an add-tree/reduce.  exp(g_b) is folded in as
  a broadcast multiply before the final softmax.

v3 engine strategy (from perfetto analysis: v1 58.1us -> v2 46.6us;
v2 = 7us startup + 28.6us packed-DVE stretch + 5us tail):
  - scores/softmax on VectorE in bf16, hot ops in 2x mode (step-1
    innermost APs; broadcasts only on middle dims); strided reduce_sum
    replaced by bf16 add-trees; reciprocal_approx_fast (~5x vs reciprocal).
  - ghv evacuated PSUM->SBUF transposed to [e, j, h] so the logits mul
    runs 2x with a contiguous reduce.
  - software pipelining ACROSS tile groups {4,4,5}: DVE emission order
    A0 A1 C0 A2 C1 E0 C2a E1 C2b E2a E2b (A=scores, C=softmax+logits,
    E=final softmax; last group's C/D/E split 3+2 to shorten the serial
    tail) so the engine FIFO never stalls on ScalarE exp handoffs.
  - startup: wa packed into the FIRST x-load chunk (one DMA gates the
    first matmul), first load is a single tile; loads {1,3,2,2,2,3}
    from a group-blocked dram layout, later loads gated on compute.
  - output stores on the Sync HWDGE queue (gpsimd SWDGE drain cost
    1.8us in the postamble).
8 NeuronCores data-parallel over batch; no cross-device communication.
"""

import numpy as np
import ml_dtypes

import concourse.bass as bass
import concourse.mybir as mybir
import concourse.tile as tile
from concourse.bacc import Bacc
from concourse.bass_utils import run_bass_kernel_spmd

F32 = mybir.dt.float32
BF16 = mybir.dt.bfloat16
NP_BF16 = ml_dtypes.bfloat16

# problem dims
B, T, NTOK, C = 64, 96, 207, 64
D, H, HD, K = 32, 4, 8, 6
E = 8
NCORES = 8

# per-core dims
B_SH = B // NCORES            # 8
S = B_SH * NTOK               # 1656 tokens per core
P = 128
NT = (S + P - 1) // P         # 13 tiles
S_PAD = NT * P                # 1664
CA = C + 1                    # 65: channels + ones row
E3 = 3 * D                    # 96 matmul output cols: q(32)|k(32)|ghv(32)
KK = K * K                    # 36
GQ = 5                        # max tiles per group (groups are 4,4,5)
TKP = K * P                   # elems per tile in the packed x layout

GROUPS = [(0, 4), (4, 4), (8, 5)]
LOADS = [(0, 1), (1, 3), (4, 2), (6, 2), (8, 2), (10, 3)]


def _build_module():
    nc = Bacc()

    # xt packs wa (96 cols) followed by NT tiles of [K, P] bf16 features
    xt = nc.dram_tensor("xt", [CA, E3 + NT * TKP], BF16, kind="ExternalInput")
    cf = nc.dram_tensor("cf", [P, E], F32, kind="ExternalInput")  # exp(g_b)
    # out[p, t, e]: token (t*128+p); host reassembles.
    out = nc.dram_tensor("out", [P, NT, E], F32, kind="ExternalOutput")

    AF = mybir.ActivationFunctionType
    AX = mybir.AxisListType

    def apv(t, dims, extra_offset=0):
        # custom AP over tile t: keep t's partition dim, replace free dims
        return bass.AP(
            tensor=t.tensor,
            offset=t.offset + extra_offset,
            ap=[list(t.ap[0])] + [list(d) for d in dims],
        )

    with tile.TileContext(nc) as tc:
        with (
            tc.tile_pool(name="singles", bufs=1) as singles,
            tc.tile_pool(name="xload", bufs=1) as xload,
            tc.tile_pool(name="work", bufs=3) as work,
            tc.tile_pool(name="psum", bufs=3, space="PSUM") as psum,
        ):
            cf_sb = singles.tile([P, E], F32)
            nc.scalar.dma_start(out=cf_sb, in_=cf[:, :])
            dum = singles.tile([P, 1], F32)
            nc.scalar.activation(out=dum, in_=cf_sb[:, 0:1], func=AF.Exp)

            xg_tiles = []
            xg_dmas = []
            for li, (ts, nl) in enumerate(LOADS):
                if li == 0:
                    xg_sb = xload.tile([CA, E3 + TKP], BF16, name="xg0")
                    xd = nc.sync.dma_start(out=xg_sb, in_=xt[:, 0:E3 + TKP])
                else:
                    xg_sb = xload.tile([CA, nl, K, P], BF16, name=f"xg{li}")
                    xd = nc.sync.dma_start(
                        out=xg_sb,
                        in_=xt[:, E3 + ts * TKP:E3 + (ts + nl) * TKP])
                xg_tiles.append(xg_sb)
                xg_dmas.append(xd)
            wa_sb = xg_tiles[0][:, 0:E3]

            out_sb = singles.tile([P, NT, E], F32)

            # ---- per-group tiles (bufs=3 -> one set per group) ----
            def group_tiles(gi):
                t = {}
                t["qk"] = work.tile([P, GQ, K, 2 * D], BF16, name="qk")
                t["gb"] = work.tile([P, GQ, E, K, H], BF16, name="gb")
                t["tmp"] = work.tile([P, GQ, KK, D], BF16, name="tmp")
                t["s1"] = work.tile([P, GQ, KK, H, 4], BF16, name="s1")
                t["s2"] = work.tile([P, GQ, KK, H, 2], BF16, name="s2")
                t["sc"] = work.tile([P, GQ, KK, H], F32, name="sc")
                t["es"] = work.tile([P, GQ, K, K, H], BF16, name="es")
                t["zt1"] = work.tile([P, GQ, K, 3, H], BF16, name="zt1")
                t["zs2"] = work.tile([P, GQ, K, H], BF16, name="zs2")
                t["zs"] = work.tile([P, GQ, K, H], F32, name="zs")
                t["rs32"] = work.tile([P, GQ, K, H], F32, name="rs32")
                t["rs16"] = work.tile([P, GQ, K, H], BF16, name="rs16")
                t["at"] = work.tile([P, GQ, K, K, H], BF16, name="at")
                t["wb1"] = work.tile([P, GQ, 3, K, H], BF16, name="wb1")
                t["wb2"] = work.tile([P, GQ, K, H], BF16, name="wb2")
                t["wbar"] = work.tile([P, GQ, K, H], BF16, name="wbar")
                t["lg1"] = work.tile([P, GQ, E, K, H], BF16, name="lg1")
                t["lgt"] = work.tile([P, GQ, E, 12], BF16, name="lgt")
                t["lg"] = work.tile([P, GQ, E], F32, name="lg")
                t["el"] = work.tile([P, GQ, E], F32, name="el")
                t["el2"] = work.tile([P, GQ, E], F32, name="el2")
                t["zf"] = work.tile([P, GQ], F32, name="zf")
                t["rf"] = work.tile([P, GQ], F32, name="rf")
                return t

            gts = [group_tiles(gi) for gi in range(len(GROUPS))]
            first_mm = {}           # global tile idx -> first matmul inst

            def tile_load(tix):
                for li, (ts, nl) in enumerate(LOADS):
                    if ts <= tix < ts + nl:
                        return li, tix - ts
                raise AssertionError

            def lhsT_of(tix, i):
                li, lidx = tile_load(tix)
                if li == 0:
                    return apv(xg_tiles[0], [[1, P]], E3 + i * P)
                return xg_tiles[li][:, lidx, i, :]

            def emit_mm_evac(gi):
                tg, g = GROUPS[gi]
                t = gts[gi]
                for u in range(g):
                    tix = tg + u
                    qkv_ps = psum.tile([P, 8, P], F32, tag="qkv_ps", bufs=3,
                                       name="qkv_ps")
                    for i in range(K):
                        mm = nc.tensor.matmul(
                            out=qkv_ps[:, i, 0:E3],
                            lhsT=lhsT_of(tix, i),
                            rhs=wa_sb,
                            start=True,
                            stop=True,
                        )
                        if i == 0 and tix not in first_mm:
                            first_mm[tix] = mm
                    # evac q,k as-is; ghv transposed to [e, j, h] so the
                    # logits mul runs in 2x mode with a contiguous reduce
                    nc.scalar.copy(out=t["qk"][:, u],
                                   in_=qkv_ps[:, 0:K, 0:2 * D])
                    nc.scalar.copy(
                        out=apv(t["gb"], [[K * H, E], [H, K], [1, H]],
                                u * E * K * H),
                        in_=apv(qkv_ps, [[1, E], [P, K], [E, H]], 2 * D))

            def emit_A(gi):
                # scores: tmp mul per tile + bf16 add tree (s1,s2 2x; s3 1x)
                tg, g = GROUPS[gi]
                t = gts[gi]
                gKK = g * KK
                for u in range(g):
                    off = u * K * 2 * D
                    q_ap = apv(t["qk"], [[2 * D, K], [0, K], [1, D]], off)
                    k_ap = apv(t["qk"], [[0, K], [2 * D, K], [1, D]], off + D)
                    tm_o = apv(t["tmp"], [[D, KK], [1, D]], u * KK * D)
                    nc.vector.tensor_mul(tm_o, q_ap, k_ap)
                a = apv(t["tmp"], [[D, gKK], [HD, H], [1, 4]])
                b = apv(t["tmp"], [[D, gKK], [HD, H], [1, 4]], 4)
                o = apv(t["s1"], [[16, gKK], [4, H], [1, 4]])
                nc.vector.tensor_add(o, a, b)
                a = apv(t["s1"], [[16, gKK], [4, H], [1, 2]])
                b = apv(t["s1"], [[16, gKK], [4, H], [1, 2]], 2)
                o = apv(t["s2"], [[8, gKK], [2, H], [1, 2]])
                nc.vector.tensor_add(o, a, b)
                a = apv(t["s2"], [[8, gKK], [2, H]])
                b = apv(t["s2"], [[8, gKK], [2, H]], 1)
                o = apv(t["sc"], [[H, gKK], [1, H]])
                nc.vector.tensor_add(o, a, b)

            def emit_B(gi):
                tg, g = GROUPS[gi]
                t = gts[gi]
                nc.scalar.activation(out=t["es"][:, 0:g], in_=t["sc"][:, 0:g],
                                     func=AF.Exp)

            def emit_C(gi, u0, ng):
                # softmax + logits for tiles [u0, u0+ng) of group gi
                t = gts[gi]
                gK = ng * K
                # zs = sum_j es via bf16 tree (reduce would read stride-4)
                a = apv(t["es"], [[24, gK], [4, 3], [1, H]], u0 * 144)
                b = apv(t["es"], [[24, gK], [4, 3], [1, H]], u0 * 144 + 12)
                o = apv(t["zt1"], [[12, gK], [4, 3], [1, H]], u0 * 72)
                nc.vector.tensor_add(o, a, b)
                a = apv(t["zt1"], [[12, gK], [1, H]], u0 * 72)
                b = apv(t["zt1"], [[12, gK], [1, H]], u0 * 72 + 4)
                o = apv(t["zs2"], [[4, gK], [1, H]], u0 * 24)
                nc.vector.tensor_add(o, a, b)
                a = apv(t["zs2"], [[4, gK], [1, H]], u0 * 24)
                b = apv(t["zt1"], [[12, gK], [1, H]], u0 * 72 + 8)
                o = apv(t["zs"], [[4, gK], [1, H]], u0 * 24)
                nc.vector.tensor_add(o, a, b)
                n = gK * H
                nc.vector.reciprocal_approx_fast(
                    out=apv(t["rs32"], [[1, n]], u0 * 24),
                    in_=apv(t["zs"], [[1, n]], u0 * 24))
                nc.scalar.copy(out=apv(t["rs16"], [[1, n]], u0 * 24),
                               in_=apv(t["rs32"], [[1, n]], u0 * 24))
                # attn = es * rs (2x: broadcast only on middle dim)
                a = apv(t["es"], [[24, gK], [4, K], [1, H]], u0 * 144)
                b = apv(t["rs16"], [[4, gK], [0, K], [1, H]], u0 * 24)
                o = apv(t["at"], [[24, gK], [4, K], [1, H]], u0 * 144)
                nc.vector.tensor_mul(o, a, b)
                # wbar = sum_i attn via bf16 tree
                a = apv(t["at"], [[144, ng], [24, 3], [1, K * H]], u0 * 144)
                b = apv(t["at"], [[144, ng], [24, 3], [1, K * H]],
                        u0 * 144 + 72)
                o = apv(t["wb1"], [[72, ng], [24, 3], [1, K * H]], u0 * 72)
                nc.vector.tensor_add(o, a, b)
                a = apv(t["wb1"], [[72, ng], [1, K * H]], u0 * 72)
                b = apv(t["wb1"], [[72, ng], [1, K * H]], u0 * 72 + 24)
                o = apv(t["wb2"], [[24, ng], [1, K * H]], u0 * 24)
                nc.vector.tensor_add(o, a, b)
                a = apv(t["wb2"], [[24, ng], [1, K * H]], u0 * 24)
                b = apv(t["wb1"], [[72, ng], [1, K * H]], u0 * 72 + 48)
                o = apv(t["wbar"], [[24, ng], [1, K * H]], u0 * 24)
                nc.vector.tensor_add(o, a, b)
                # logits partial: lg1[g, e, (j,h)] = wbar[g, (j,h)] * ghv
                a = apv(t["wbar"], [[24, ng], [0, E], [1, K * H]], u0 * 24)
                b = apv(t["gb"], [[K * H * E, ng], [K * H, E], [1, K * H]],
                        u0 * 192)
                o = apv(t["lg1"], [[K * H * E, ng], [K * H, E], [1, K * H]],
                        u0 * 192)
                nc.vector.tensor_mul(o, a, b)
                # half-tree then contiguous reduce over 12
                a = apv(t["lg1"], [[K * H * E, ng], [K * H, E], [1, 12]],
                        u0 * 192)
                b = apv(t["lg1"], [[K * H * E, ng], [K * H, E], [1, 12]],
                        u0 * 192 + 12)
                o = apv(t["lgt"], [[12 * E, ng], [12, E], [1, 12]], u0 * 96)
                nc.vector.tensor_add(o, a, b)
                nc.vector.reduce_sum(
                    out=apv(t["lg"], [[E, ng], [1, E]], u0 * E),
                    in_=apv(t["lgt"], [[12 * E, ng], [12, E], [1, 12]],
                            u0 * 96),
                    axis=AX.X)

            def emit_D(gi, u0, ng):
                t = gts[gi]
                nc.scalar.activation(out=t["el"][:, u0:u0 + ng],
                                     in_=t["lg"][:, u0:u0 + ng], func=AF.Exp)

            def emit_E(gi, u0, ng):
                tg, g = GROUPS[gi]
                t = gts[gi]
                ebg = apv(cf_sb, [[0, ng], [1, E]])
                nc.vector.tensor_mul(t["el2"][:, u0:u0 + ng],
                                     t["el"][:, u0:u0 + ng], ebg)
                nc.vector.reduce_sum(out=t["zf"][:, u0:u0 + ng],
                                     in_=t["el2"][:, u0:u0 + ng], axis=AX.X)
                nc.vector.reciprocal_approx_fast(out=t["rf"][:, u0:u0 + ng],
                                                 in_=t["zf"][:, u0:u0 + ng])
                rf_ap = apv(t["rf"], [[1, ng], [0, E]], u0)
                nc.vector.tensor_mul(out_sb[:, tg + u0:tg + u0 + ng, :],
                                     t["el2"][:, u0:u0 + ng], rf_ap)
                nc.sync.dma_start(out=out[:, tg + u0:tg + u0 + ng, :],
                                  in_=out_sb[:, tg + u0:tg + u0 + ng, :])

            # ---- software-pipelined emission ----
            emit_mm_evac(0)
            emit_mm_evac(1)
            emit_A(0)
            emit_A(1)
            emit_B(0)
            emit_mm_evac(2)
            emit_C(0, 0, 4)
            emit_A(2)
            emit_B(1)
            emit_C(1, 0, 4)
            emit_D(0, 0, 4)
            emit_E(0, 0, 4)
            emit_B(2)
            emit_C(2, 0, 3)
            emit_D(1, 0, 4)
            emit_E(1, 0, 4)
            emit_D(2, 0, 3)
            emit_C(2, 3, 2)
            emit_E(2, 0, 3)
            emit_D(2, 3, 2)
            emit_E(2, 3, 2)

            # later loads wait on compute progress so early tiles don't
            # round-robin behind all the loads
            for li in range(2, len(LOADS)):
                gate_tile = LOADS[li - 2][0]
                tile.add_dep_helper(xg_dmas[li].ins, first_mm[gate_tile].ins,
                                    sync=True, reason="load stagger")

    nc.finalize()
    return nc


_NC = None


def _get_module():
    global _NC
    if _NC is None:
        _NC = _build_module()
    return _NC


def _host_prep(x, proj_w, proj_b, in_proj_w, in_proj_b, out_w, out_b, fc_w, fc_b):
    scale = np.float32(1.0 / np.sqrt(HD))
    w_eff = (in_proj_w @ proj_w).astype(np.float32)          # [96, 64]
    b_eff = (in_proj_w @ proj_b + in_proj_b).astype(np.float32)
    w_eff[0:D] *= scale
    b_eff[0:D] *= scale
    w_aug = np.concatenate([w_eff, b_eff[:, None]], axis=1)  # [96, 65]

    G = (fc_w @ out_w / np.float32(K)).astype(np.float32)    # [8, 32]
    g_b = (fc_w @ out_b + fc_b).astype(np.float32)

    wa = np.zeros((CA, E3), dtype=np.float32)
    wa[:, 0:2 * D] = w_aug[0:2 * D].T                        # q | k
    for h in range(H):
        wv_h = w_aug[2 * D + HD * h:2 * D + HD * (h + 1)]    # [8, 65]
        G_h = G[:, HD * h:HD * (h + 1)]                      # [8(e), 8(c)]
        wa[:, 2 * D + E * h:2 * D + E * (h + 1)] = wv_h.T @ G_h.T
    wa = wa.astype(NP_BF16)

    cf = np.broadcast_to(np.exp(g_b).astype(np.float32)[None, :],
                         (P, E)).copy()

    # x: [B, T, N, C] -> last K steps -> per-core packed [CA, 96 + NT*K*P]
    xk = x[:, T - K:, :, :]                                  # [B, K, N, C]
    in_maps = []
    for core in range(NCORES):
        xc = xk[core * B_SH:(core + 1) * B_SH]               # [8, K, N, C]
        xc = np.transpose(xc, (3, 1, 0, 2)).reshape(C, K, S)
        xp = np.zeros((C, K, S_PAD), dtype=np.float32)
        xp[:, :, 0:S] = xc
        xp = xp.reshape(C, K, NT, P).transpose(0, 2, 1, 3)   # [C, NT, K, P]
        xtc = np.empty((CA, E3 + NT * TKP), dtype=NP_BF16)
        xtc[:, 0:E3] = wa
        xfull = np.empty((CA, NT, K, P), dtype=NP_BF16)
        xfull[0:C] = xp.astype(NP_BF16)
        xfull[C] = 1
        xtc[:, E3:] = xfull.reshape(CA, NT * TKP)
        in_maps.append({"xt": xtc, "cf": cf})
    return in_maps


def kernel(x, proj_w, proj_b, in_proj_w, in_proj_b, out_w, out_b, fc_w, fc_b,
           _trace=False):
    in_maps = _host_prep(np.asarray(x, dtype=np.float32),
                         np.asarray(proj_w, dtype=np.float32),
                         np.asarray(proj_b, dtype=np.float32),
                         np.asarray(in_proj_w, dtype=np.float32),
                         np.asarray(in_proj_b, dtype=np.float32),
                         np.asarray(out_w, dtype=np.float32),
                         np.asarray(out_b, dtype=np.float32),
                         np.asarray(fc_w, dtype=np.float32),
                         np.asarray(fc_b, dtype=np.float32))
    nc = _get_module()
    res = run_bass_kernel_spmd(nc, in_maps, core_ids=list(range(NCORES)),
                               trace=_trace)
    outs = []
    for core in range(NCORES):
        oc = res.results[core]["out"]                        # [P, NT, E]
        oc = oc.transpose(1, 0, 2).reshape(S_PAD, E)[:S]
        oc = oc.reshape(B_SH, NTOK, E)
        outs.append(oc)
    full = np.concatenate(outs, axis=0)                      # [64, 207, 8]
    if _trace:
        kernel._last_exec_time_ns = res.exec_time_ns
        kernel._last_profile = res.profile_json
    return full.astype(np.float32)
